# revision 1
# baseline (speedup 1.0000x reference)
"""Trainium2 Bass kernel for nn_Dewarp (cost-volume argmin dewarp).

Reference semantics (fp32):
    for each offset (dx outer, dy inner) in [-6,6]^2  (169 offsets):
        tmp  = sqrt(sum_c (A[c, h+dy, w+dx] - B[c, h, w])^2)     (A zero-padded)
        w    = avg_11x11(tmp)          (zero-padded box filter)
        mask = w_run >= w ; res = where(mask, tmp, res); w_run = where(mask, w, w_run)
    out = 3x3 min-pool of res (+inf padded)

Layout on chip: partitions = W columns (tiles of 128), free dim = H rows.
Sharding: H split across 8 cores (192 output rows each, halo via host slicing).

Engine split per offset: GPSIMD does channel diffs + first box-H add, ACT does
squares + sqrt, DVE does sums / rest of box-H tree / compare-select, PE does the
11-wide box filter along W as a band matmul into PSUM.
"""

import sys

for _p in ("/opt/trn_rl_repo", "/root/.axon_site/_ro/trn_rl_repo"):
    if _p not in sys.path:
        sys.path.append(_p)

import numpy as np

import concourse.bass as bass
import concourse.tile as tile
from concourse import bacc, mybir
from concourse.bass_utils import run_bass_kernel_spmd

F32 = mybir.dt.float32
OFF = 6          # max |offset| actually used (D-1 in reference)
BOX = 5          # box filter half-width (KS//2)
BIG = 1.0e30     # stand-in for +inf padding


class Geom:
    def __init__(self, W, out_h, n_cores):
        assert W % 128 == 0
        self.W = W
        self.T = W // 128          # number of 128-col partition tiles
        self.OUT_H = out_h         # output rows per core
        self.RES_H = out_h + 2     # res rows (minpool halo)
        self.TMP_H = out_h + 12    # tmp rows (res +- BOX)
        self.A_H = out_h + 24      # A rows   (tmp +- OFF)
        self.A_W = W + 12          # A cols   (+- OFF)
        self.B_H = out_h + 12
        self.n_cores = n_cores
        self.H = out_h * n_cores   # full image height
        # consts: 3 band mats | top/bot halo masks | BIG row for minpool edges
        self.CW = 384 + 2 * self.T + out_h


GEOM = Geom(W=1536, out_h=192, n_cores=8)


def make_consts(g: Geom, core: int) -> np.ndarray:
    """[128, CW]: band matrix, edge triangles, minpool-halo masks."""
    c = np.zeros((128, g.CW), dtype=np.float32)
    # band weight 1.0: w is compared, never read — raw box sums order the same
    # way as the reference's sum/121 (division by a positive constant is
    # monotone), and skipping the scale removes a per-element rounding.
    v = np.float32(1.0)
    k = np.arange(128)
    p = np.arange(128)
    c[:, 0:128] = (np.abs(k[:, None] - p[None, :]) <= 5) * v      # band_main[k,p]
    # bandL: output cols p<=4 of tile t pull rows 123..127 of tile t-1
    # (col p sums neighbours p-5..p-1 => rows 128+p-5 .. 127 of prev tile)
    c[:, 128:256] = ((k[:, None] - 128 >= p[None, :] - 5)
                     & (k[:, None] >= 123) & (p[None, :] <= 4)) * v
    # bandR: output cols p>=123 pull rows 0..4 of tile t+1
    c[:, 256:384] = ((k[:, None] + 128 <= p[None, :] + 5)
                     & (k[:, None] <= 4) & (p[None, :] >= 123)) * v
    if core == 0:
        c[:, 384:384 + g.T] = BIG                                  # top halo row invalid
    if core == g.n_cores - 1:
        c[:, 384 + g.T:384 + 2 * g.T] = BIG                        # bottom halo row invalid
    c[:, 384 + 2 * g.T:] = BIG                                     # minpool edge fill
    return c


def host_prepare(A: np.ndarray, B: np.ndarray, g: Geom):
    """A,B: [1,3,H,W] fp32 -> per-core input maps (W-major stripes)."""
    A = np.asarray(A, dtype=np.float32).reshape(3, g.H, g.W)
    B = np.asarray(B, dtype=np.float32).reshape(3, g.H, g.W)
    Apad = np.zeros((3, g.H + 24, g.W + 12), dtype=np.float32)
    Apad[:, 12:12 + g.H, 6:6 + g.W] = A
    Bpad = np.zeros((3, g.H + 12, g.W), dtype=np.float32)
    Bpad[:, 6:6 + g.H, :] = B
    in_maps = []
    for i in range(g.n_cores):
        r0 = i * g.OUT_H
        a = np.ascontiguousarray(
            Apad[:, r0:r0 + g.A_H, :].transpose(0, 2, 1))   # [3, A_W, A_H]
        b = np.ascontiguousarray(
            Bpad[:, r0:r0 + g.B_H, :].transpose(0, 2, 1))   # [3, W, B_H]
        in_maps.append({"A_t": a, "B_t": b, "CONST": make_consts(g, i)})
    return in_maps


def host_assemble(outs, g: Geom) -> np.ndarray:
    full = np.empty((g.W, g.H), dtype=np.float32)
    for i, om in enumerate(outs):
        full[:, i * g.OUT_H:(i + 1) * g.OUT_H] = om["OUT"]
    return np.ascontiguousarray(full.T).reshape(1, 1, g.H, g.W)


def build_body(nc, tc, in_aps, out_ap, g: Geom, pair_matmuls=False):
    """Emit the kernel body inside an active TileContext.

    in_aps: dict name -> DRAM AP for A_t [3,A_W,A_H], B_t [3,W,B_H], CONST [128,CW]
    out_ap: DRAM AP [W, OUT_H]
    """
    T, TMP_H, RES_H, OUT_H, A_H, B_H = g.T, g.TMP_H, g.RES_H, g.OUT_H, g.A_H, g.B_H
    A_d, B_d, C_d = in_aps["A_t"], in_aps["B_t"], in_aps["CONST"]

    assert T % 2 == 0
    persist = tc.alloc_tile_pool(name="persist", bufs=1)
    apool = tc.alloc_tile_pool(name="apool", bufs=2)
    tmpp = tc.alloc_tile_pool(name="tmpp", bufs=2)
    scrp = tc.alloc_tile_pool(name="scr", bufs=3)
    psump = tc.alloc_tile_pool(name="psum", bufs=1, space="PSUM")

    const = persist.tile([128, g.CW], F32, tag="const")
    nc.sync.dma_start(out=const[:], in_=C_d[:, :])
    band_main = const[:, 0:128]
    band_l = const[:, 128:256]
    band_r = const[:, 256:384]

    B_sb = persist.tile([128, 3 * T * B_H], F32, tag="B")
    B4 = B_sb[:].rearrange("p (c t h) -> p c t h", c=3, t=T)
    for c in range(3):
        nc.sync.dma_start(
            out=B4[:, c, :, :],
            in_=B_d[c].rearrange("(t p) h -> p t h", p=128))

    # stride-padded so 3D views stay 3D in the interp (contiguous dims collapse)
    RES_HP = RES_H + 2
    res = persist.tile([128, T * RES_HP], F32, tag="res")
    w_run = persist.tile([128, T * RES_HP], F32, tag="wrun")
    res3 = res[:].rearrange("p (t h) -> p t h", t=T)[:, :, 0:RES_H]
    wrun3 = w_run[:].rearrange("p (t h) -> p t h", t=T)[:, :, 0:RES_H]

    # box-H output: persistent, T+2 tiles (stride RES_HP) with zeroed guard
    # tiles at both ends so edge band matmuls can pair tiles uniformly; the
    # 2-col pads stay zero so matmul over a full 2*RES_HP-wide pair is safe
    th = persist.tile([128, (T + 2) * RES_HP], F32, tag="th")
    th3z = th[:].rearrange("p (t h) -> p t h", t=T + 2)
    th3 = th3z[:, 1:T + 1, 0:RES_H]
    nc.vector.memset(th[:], 0.0)
    mask = persist.tile([128, T * RES_HP], mybir.dt.uint8, tag="mask")
    mask3 = mask[:].rearrange("p (t h) -> p t h", t=T)[:, :, 0:RES_H]

    def v3(tile_, n):  # [128, T*n] view as [128, T, n] using tile's full stride
        return tile_[:].rearrange("p (t h) -> p t h", t=T)

    oi = 0
    for dx in range(-OFF, OFF + 1):
        # column-shifted A stripe for this dx (all 3 channels, one DMA)
        at = apool.tile([128, 3 * T * A_H], F32, tag="A")
        A4 = at[:].rearrange("p (c t h) -> p c t h", c=3, t=T)
        for c in range(3):
            nc.sync.dma_start(
                out=A4[:, c, :, :],
                in_=A_d[c, OFF + dx:OFF + dx + g.W, :].rearrange(
                    "(t p) h -> p t h", p=128))

        for dy in range(-OFF, OFF + 1):
            d0 = tmpp.tile([128, T * TMP_H], F32, tag="tmp")
            d1 = scrp.tile([128, T * TMP_H], F32, tag="scr")
            d2 = scrp.tile([128, T * TMP_H], F32, tag="scr")
            d0v, d1v, d2v = v3(d0, TMP_H), v3(d1, TMP_H), v3(d2, TMP_H)

            def ash(c):
                return A4[:, c, :, OFF + dy:OFF + dy + TMP_H]

            def bsh(c):
                return B4[:, c, :, 0:TMP_H]

            # d_c = A_c(shifted) - B_c ; s = d0^2 + d1^2 + d2^2 ; tmp = sqrt(s)
            nc.gpsimd.tensor_tensor(d0v, ash(0), bsh(0), mybir.AluOpType.subtract)
            nc.gpsimd.tensor_tensor(d1v, ash(1), bsh(1), mybir.AluOpType.subtract)
            nc.gpsimd.tensor_tensor(d2v, ash(2), bsh(2), mybir.AluOpType.subtract)
            nc.scalar.activation(d0v, d0v, mybir.ActivationFunctionType.Square)
            nc.scalar.activation(d1v, d1v, mybir.ActivationFunctionType.Square)
            nc.scalar.activation(d2v, d2v, mybir.ActivationFunctionType.Square)
            nc.gpsimd.tensor_tensor(d0v, d0v, d1v, mybir.AluOpType.add)
            nc.vector.tensor_tensor(d0v, d0v, d2v, mybir.AluOpType.add)
            nc.scalar.activation(d0v, d0v, mybir.ActivationFunctionType.Sqrt)
            tmp3 = d0v  # tmp lives in d0

            # box-H (11-tap sum along free dim), log tree: a=2, b=4, c=8 taps
            nc.vector.tensor_tensor(  # a -> d1
                d1v[:, :, 0:TMP_H - 1],
                tmp3[:, :, 0:TMP_H - 1], tmp3[:, :, 1:TMP_H],
                mybir.AluOpType.add)
            nc.vector.tensor_tensor(  # b -> d2
                d2v[:, :, 0:TMP_H - 3],
                d1v[:, :, 0:TMP_H - 3], d1v[:, :, 2:TMP_H - 1],
                mybir.AluOpType.add)
            nc.vector.tensor_tensor(  # c -> d2 in place (positive shift: safe)
                d2v[:, :, 0:TMP_H - 7],
                d2v[:, :, 0:TMP_H - 7], d2v[:, :, 4:TMP_H - 3],
                mybir.AluOpType.add)
            nc.vector.tensor_tensor(  # th = c + a[+8]
                th3, d2v[:, :, 0:RES_H], d1v[:, :, 8:8 + RES_H],
                mybir.AluOpType.add)
            nc.vector.tensor_tensor(  # th += tmp[+10]
                th3, th3, tmp3[:, :, 10:10 + RES_H], mybir.AluOpType.add)

            # box-W via fp32 band matmuls into PSUM (per-tile slot stride
            # 256). Paired tiles on HW (fewer calls, keeps PE p-state warm);
            # the interp can't check 3D matmul outs, so sim mode is per-tile.
            w_ps = psump.tile([128, T * 256], F32, tag="wps")
            wps_t = w_ps[:].rearrange("p (t x) -> p t x", t=T)
            R = mybir.dt.float32r
            if pair_matmuls:
                for pr in range(T // 2):
                    out_sl = wps_t[:, 2 * pr:2 * pr + 2, 0:RES_HP]
                    nc.tensor.matmul(
                        out_sl, band_main,
                        th3z[:, 1 + 2 * pr:3 + 2 * pr, :],
                        start=True, stop=False)
                    nc.tensor.matmul(
                        out_sl, band_l,
                        th3z[:, 2 * pr:2 + 2 * pr, :],
                        start=False, stop=False)
                    nc.tensor.matmul(
                        out_sl, band_r,
                        th3z[:, 2 + 2 * pr:4 + 2 * pr, :],
                        start=False, stop=True)
            else:
                for t in range(T):
                    out_sl = wps_t[:, t, 0:RES_HP]
                    nc.tensor.matmul(out_sl, band_main,
                                     th3z[:, 1 + t, :],
                                     start=True, stop=False)
                    nc.tensor.matmul(out_sl, band_l,
                                     th3z[:, t, :],
                                     start=False, stop=False)
                    nc.tensor.matmul(out_sl, band_r,
                                     th3z[:, 2 + t, :],
                                     start=False, stop=True)

            wps3 = wps_t[:, :, 0:RES_H]
            tmpc = tmp3[:, :, BOX:BOX + RES_H]

            if oi == 0:
                nc.scalar.activation(res3, tmpc, mybir.ActivationFunctionType.Copy)
                nc.scalar.activation(wrun3, wps3, mybir.ActivationFunctionType.Copy)
            else:
                nc.vector.tensor_tensor(mask3, wrun3, wps3, mybir.AluOpType.is_ge)
                nc.vector.copy_predicated(res3, mask3, tmpc)
                nc.vector.tensor_tensor(wrun3, wrun3, wps3, mybir.AluOpType.min)
            oi += 1

    # minpool-halo rows outside the image -> +BIG
    top = const[:, 384:384 + T].rearrange("p (t o) -> p t o", o=1)
    bot = const[:, 384 + T:384 + 2 * T].rearrange("p (t o) -> p t o", o=1)
    nc.vector.tensor_tensor(res3[:, :, 0:1], res3[:, :, 0:1], top,
                            mybir.AluOpType.add)
    nc.vector.tensor_tensor(res3[:, :, RES_H - 1:RES_H],
                            res3[:, :, RES_H - 1:RES_H], bot,
                            mybir.AluOpType.add)

    # 3x3 min-pool: H direction (free dim)
    m = scrp.tile([128, T * TMP_H], F32, tag="scr")
    m3 = v3(m, TMP_H)[:, :, 0:OUT_H]
    nc.vector.tensor_tensor(m3, res3[:, :, 0:OUT_H], res3[:, :, 1:1 + OUT_H],
                            mybir.AluOpType.min)
    nc.vector.tensor_tensor(m3, m3, res3[:, :, 2:2 + OUT_H], mybir.AluOpType.min)

    # W direction: shift columns +-1 by round-tripping through a DRAM staging
    # buffer (SBUF access patterns must start at partition 0/32/64/96, so a
    # partition shift cannot be expressed on-chip).
    dpool = tc.alloc_tile_pool(name="dramstage", bufs=1, space="DRAM")
    stage = dpool.tile([g.W + 2, OUT_H], F32, tag="stage")
    bigrow = const[0:1, 384 + 2 * T:384 + 2 * T + OUT_H]
    nc.sync.dma_start(out=stage[0:1, :], in_=bigrow)
    nc.sync.dma_start(out=stage[g.W + 1:g.W + 2, :], in_=bigrow)
    nc.sync.dma_start(
        out=stage[1:1 + g.W, :].rearrange("(t p) h -> p t h", p=128), in_=m3)
    msp = scrp.tile([128, T * TMP_H], F32, tag="scr")   # m[w+1]
    msm = scrp.tile([128, T * TMP_H], F32, tag="scr")   # m[w-1]
    msp3 = v3(msp, TMP_H)[:, :, 0:OUT_H]
    msm3 = v3(msm, TMP_H)[:, :, 0:OUT_H]
    nc.sync.dma_start(
        out=msp3, in_=stage[2:2 + g.W, :].rearrange("(t p) h -> p t h", p=128))
    nc.sync.dma_start(
        out=msm3, in_=stage[0:g.W, :].rearrange("(t p) h -> p t h", p=128))
    nc.vector.tensor_tensor(m3, m3, msp3, mybir.AluOpType.min)
    nc.vector.tensor_tensor(m3, m3, msm3, mybir.AluOpType.min)
    dpool.release()

    nc.sync.dma_start(
        out=out_ap.rearrange("(t p) h -> p t h", p=128), in_=m3)

    for p in (psump, scrp, tmpp, apool, persist):
        p.release()


def make_program(g: Geom):
    nc = bacc.Bacc("TRN2", target_bir_lowering=False, debug=False,
                   num_devices=g.n_cores)
    A_d = nc.dram_tensor("A_t", [3, g.A_W, g.A_H], F32, kind="ExternalInput")
    B_d = nc.dram_tensor("B_t", [3, g.W, g.B_H], F32, kind="ExternalInput")
    C_d = nc.dram_tensor("CONST", [128, g.CW], F32, kind="ExternalInput")
    O_d = nc.dram_tensor("OUT", [g.W, g.OUT_H], F32, kind="ExternalOutput")
    with tile.TileContext(nc) as tc:
        build_body(nc, tc,
                   {"A_t": A_d.ap(), "B_t": B_d.ap(), "CONST": C_d.ap()},
                   O_d.ap(), g, pair_matmuls=True)
    nc.finalize()
    return nc


_NC_CACHE = {}
_LAST = {}


def kernel(A, B):
    g = GEOM
    key = (g.W, g.OUT_H, g.n_cores)
    if key not in _NC_CACHE:
        _NC_CACHE[key] = make_program(g)
    nc = _NC_CACHE[key]
    in_maps = host_prepare(A, B, g)
    r = run_bass_kernel_spmd(nc, in_maps, list(range(g.n_cores)))
    _LAST["r"] = r
    return host_assemble(r.results, g)


if __name__ == "__main__":
    a = np.random.randn(1, 3, GEOM.H, GEOM.W).astype(np.float32)
    b = np.random.randn(1, 3, GEOM.H, GEOM.W).astype(np.float32)
    out = kernel(a, b)
    print("out", out.shape, out.dtype, float(out.mean()))



# revision 4
# speedup vs baseline: 1.2244x; 1.2244x over previous
"""Trainium2 Bass kernel for nn_Dewarp (cost-volume argmin dewarp).

Reference semantics (fp32):
    for each offset (dx outer, dy inner) in [-6,6]^2  (169 offsets):
        tmp  = sqrt(sum_c (A[c, h+dy, w+dx] - B[c, h, w])^2)     (A zero-padded)
        w    = avg_11x11(tmp)          (zero-padded box filter)
        mask = w_run >= w ; res = where(mask, tmp, res); w_run = where(mask, w, w_run)
    out = 3x3 min-pool of res (+inf padded)

Layout on chip: partitions = W columns (tiles of 128), free dim = H rows.
Sharding: H split across 8 cores (192 output rows each, halo via host slicing).

Engine split per offset: GPSIMD does channel diffs + first box-H add, ACT does
squares + sqrt, DVE does sums / rest of box-H tree / compare-select, PE does the
11-wide box filter along W as a band matmul into PSUM.
"""

import sys

for _p in ("/opt/trn_rl_repo", "/root/.axon_site/_ro/trn_rl_repo"):
    if _p not in sys.path:
        sys.path.append(_p)

import numpy as np

import concourse.bass as bass
import concourse.tile as tile
from concourse import bacc, mybir
from concourse.bass_utils import run_bass_kernel_spmd

F32 = mybir.dt.float32
OFF = 6          # max |offset| actually used (D-1 in reference)
BOX = 5          # box filter half-width (KS//2)
BIG = 1.0e30     # stand-in for +inf padding


class Geom:
    def __init__(self, W, out_h, n_cores):
        assert W % 128 == 0
        self.W = W
        self.T = W // 128          # number of 128-col partition tiles
        self.OUT_H = out_h         # output rows per core
        self.RES_H = out_h + 2     # res rows (minpool halo)
        self.TMP_H = out_h + 12    # tmp rows (res +- BOX)
        self.A_H = out_h + 24      # A rows   (tmp +- OFF)
        self.A_W = W + 12          # A cols   (+- OFF)
        self.B_H = out_h + 12
        self.n_cores = n_cores
        self.H = out_h * n_cores   # full image height
        # consts: 3 band mats | top/bot halo masks | BIG row for minpool edges
        self.CW = 384 + 2 * self.T + out_h


GEOM = Geom(W=1536, out_h=192, n_cores=8)


def make_consts(g: Geom, core: int) -> np.ndarray:
    """[128, CW]: band matrix, edge triangles, minpool-halo masks."""
    c = np.zeros((128, g.CW), dtype=np.float32)
    # band weight 1.0: w is compared, never read — raw box sums order the same
    # way as the reference's sum/121 (division by a positive constant is
    # monotone), and skipping the scale removes a per-element rounding.
    v = np.float32(1.0)
    k = np.arange(128)
    p = np.arange(128)
    c[:, 0:128] = (np.abs(k[:, None] - p[None, :]) <= 5) * v      # band_main[k,p]
    # bandL: output cols p<=4 of tile t pull rows 123..127 of tile t-1
    # (col p sums neighbours p-5..p-1 => rows 128+p-5 .. 127 of prev tile)
    c[:, 128:256] = ((k[:, None] - 128 >= p[None, :] - 5)
                     & (k[:, None] >= 123) & (p[None, :] <= 4)) * v
    # bandR: output cols p>=123 pull rows 0..4 of tile t+1
    c[:, 256:384] = ((k[:, None] + 128 <= p[None, :] + 5)
                     & (k[:, None] <= 4) & (p[None, :] >= 123)) * v
    if core == 0:
        c[:, 384:384 + g.T] = BIG                                  # top halo row invalid
    if core == g.n_cores - 1:
        c[:, 384 + g.T:384 + 2 * g.T] = BIG                        # bottom halo row invalid
    c[:, 384 + 2 * g.T:] = BIG                                     # minpool edge fill
    return c


def host_prepare(A: np.ndarray, B: np.ndarray, g: Geom):
    """A,B: [1,3,H,W] fp32 -> per-core input maps (W-major stripes)."""
    A = np.asarray(A, dtype=np.float32).reshape(3, g.H, g.W)
    B = np.asarray(B, dtype=np.float32).reshape(3, g.H, g.W)
    Apad = np.zeros((3, g.H + 24, g.W + 12), dtype=np.float32)
    Apad[:, 12:12 + g.H, 6:6 + g.W] = A
    Bpad = np.zeros((3, g.H + 12, g.W), dtype=np.float32)
    Bpad[:, 6:6 + g.H, :] = B
    in_maps = []
    for i in range(g.n_cores):
        r0 = i * g.OUT_H
        a = np.ascontiguousarray(
            Apad[:, r0:r0 + g.A_H, :].transpose(0, 2, 1))   # [3, A_W, A_H]
        b = np.ascontiguousarray(
            Bpad[:, r0:r0 + g.B_H, :].transpose(0, 2, 1))   # [3, W, B_H]
        in_maps.append({"A_t": a, "B_t": b, "CONST": make_consts(g, i)})
    return in_maps


def host_assemble(outs, g: Geom) -> np.ndarray:
    full = np.empty((g.W, g.H), dtype=np.float32)
    for i, om in enumerate(outs):
        full[:, i * g.OUT_H:(i + 1) * g.OUT_H] = om["OUT"]
    return np.ascontiguousarray(full.T).reshape(1, 1, g.H, g.W)


def build_body(nc, tc, in_aps, out_ap, g: Geom, pair_matmuls=False):
    """Emit the kernel body inside an active TileContext.

    in_aps: dict name -> DRAM AP for A_t [3,A_W,A_H], B_t [3,W,B_H], CONST [128,CW]
    out_ap: DRAM AP [W, OUT_H]
    """
    T, TMP_H, RES_H, OUT_H, A_H, B_H = g.T, g.TMP_H, g.RES_H, g.OUT_H, g.A_H, g.B_H
    A_d, B_d, C_d = in_aps["A_t"], in_aps["B_t"], in_aps["CONST"]

    assert T % 2 == 0
    persist = tc.alloc_tile_pool(name="persist", bufs=1)
    apool = tc.alloc_tile_pool(name="apool", bufs=2)
    tmpp = tc.alloc_tile_pool(name="tmpp", bufs=2)
    scrp = tc.alloc_tile_pool(name="scr", bufs=3)
    psump = tc.alloc_tile_pool(name="psum", bufs=1, space="PSUM")

    const = persist.tile([128, g.CW], F32, tag="const")
    nc.sync.dma_start(out=const[:], in_=C_d[:, :])
    band_main = const[:, 0:128]
    band_l = const[:, 128:256]
    band_r = const[:, 256:384]

    B_sb = persist.tile([128, 3 * T * B_H], F32, tag="B")
    B4 = B_sb[:].rearrange("p (c t h) -> p c t h", c=3, t=T)
    for c in range(3):
        nc.sync.dma_start(
            out=B4[:, c, :, :],
            in_=B_d[c].rearrange("(t p) h -> p t h", p=128))

    # stride-padded so 3D views stay 3D in the interp (contiguous dims collapse)
    RES_HP = RES_H + 2
    res = persist.tile([128, T * RES_HP], F32, tag="res")
    w_run = persist.tile([128, T * RES_HP], F32, tag="wrun")
    res3 = res[:].rearrange("p (t h) -> p t h", t=T)[:, :, 0:RES_H]
    wrun3 = w_run[:].rearrange("p (t h) -> p t h", t=T)[:, :, 0:RES_H]

    # box-H output: persistent, T+2 slots of width THW; slot 1+t holds the
    # 204-long sliding-window scan output for tile t (valid box sums for res
    # row j live at col 10+j); guard slots at both ends stay zero so the edge
    # band matmuls can pair slots uniformly.
    THW = 206
    th = persist.tile([128, (T + 2) * THW], F32, tag="th")
    th3z = th[:].rearrange("p (t h) -> p t h", t=T + 2)
    nc.vector.memset(th[:], 0.0)
    mask = persist.tile([128, T * RES_HP], mybir.dt.uint8, tag="mask")
    mask3 = mask[:].rearrange("p (t h) -> p t h", t=T)[:, :, 0:RES_H]

    def v3(tile_, n):  # [128, T*n] view as [128, T, n] using tile's full stride
        return tile_[:].rearrange("p (t h) -> p t h", t=T)

    oi = 0
    for dx in range(-OFF, OFF + 1):
        # column-shifted A stripe for this dx (all 3 channels, one DMA)
        at = apool.tile([128, 3 * T * A_H], F32, tag="A")
        A4 = at[:].rearrange("p (c t h) -> p c t h", c=3, t=T)
        for c in range(3):
            nc.sync.dma_start(
                out=A4[:, c, :, :],
                in_=A_d[c, OFF + dx:OFF + dx + g.W, :].rearrange(
                    "(t p) h -> p t h", p=128))

        for dy in range(-OFF, OFF + 1):
            # d0 uses a zero-prefixed layout: cols 0..10 are zeros, cols
            # 11..11+TMP_H hold data, so a single running-window scan per tile
            # yields the 11-tap box sums with no initial value.
            TZW = 216
            d0 = tmpp.tile([128, T * TZW], F32, tag="tmp")
            d1 = scrp.tile([128, T * TMP_H], F32, tag="scr")
            d2 = scrp.tile([128, T * TMP_H], F32, tag="scr")
            d04 = d0[:].rearrange("p (t h) -> p t h", t=T)
            d0v = d04[:, :, 11:11 + TMP_H]
            d1v, d2v = v3(d1, TMP_H), v3(d2, TMP_H)
            nc.vector.memset(d04[:, :, 0:11], 0.0)

            def ash(c):
                return A4[:, c, :, OFF + dy:OFF + dy + TMP_H]

            def bsh(c):
                return B4[:, c, :, 0:TMP_H]

            # d_c = A_c(shifted) - B_c ; s = d0^2 + d1^2 + d2^2 ; tmp = sqrt(s)
            nc.gpsimd.tensor_tensor(d0v, ash(0), bsh(0), mybir.AluOpType.subtract)
            nc.vector.tensor_tensor(d1v, ash(1), bsh(1), mybir.AluOpType.subtract)
            nc.gpsimd.tensor_tensor(d2v, ash(2), bsh(2), mybir.AluOpType.subtract)
            nc.scalar.activation(d0v, d0v, mybir.ActivationFunctionType.Square)
            nc.scalar.activation(d1v, d1v, mybir.ActivationFunctionType.Square)
            nc.scalar.activation(d2v, d2v, mybir.ActivationFunctionType.Square)
            nc.gpsimd.tensor_tensor(d0v, d0v, d1v, mybir.AluOpType.add)
            nc.vector.tensor_tensor(d0v, d0v, d2v, mybir.AluOpType.add)
            nc.scalar.activation(d0v, d0v, mybir.ActivationFunctionType.Sqrt)
            tmp3 = d0v  # tmp lives in d0 cols 11..215

            # box-H: one sliding-window scan per tile:
            #   state[j] = state[j-1] + tmpz[j+11] - tmpz[j]
            # telescopes to sum(tmpz[j+1..j+11]); box for res row r is at
            # scan output index r+10.
            for t in range(T):
                nc.vector.tensor_tensor_scan(
                    th[:, (1 + t) * THW:(1 + t) * THW + TMP_H],
                    d0[:, t * TZW + 11:t * TZW + 11 + TMP_H],
                    d0[:, t * TZW:t * TZW + TMP_H],
                    0.0, mybir.AluOpType.add, mybir.AluOpType.subtract)

            # box-W via fp32 band matmuls into PSUM (per-tile slot stride
            # 256). Paired tiles on HW (fewer calls, keeps PE p-state warm);
            # the interp can't check 3D matmul outs, so sim mode is per-tile.
            w_ps = psump.tile([128, T * 256], F32, tag="wps")
            wps_t = w_ps[:].rearrange("p (t x) -> p t x", t=T)
            R = mybir.dt.float32r
            if pair_matmuls:
                for pr in range(T // 2):
                    out_sl = wps_t[:, 2 * pr:2 * pr + 2, 0:RES_HP]
                    nc.tensor.matmul(
                        out_sl, band_main,
                        th3z[:, 1 + 2 * pr:3 + 2 * pr, 10:10 + RES_HP],
                        start=True, stop=False)
                    nc.tensor.matmul(
                        out_sl, band_l,
                        th3z[:, 2 * pr:2 + 2 * pr, 10:10 + RES_HP],
                        start=False, stop=False)
                    nc.tensor.matmul(
                        out_sl, band_r,
                        th3z[:, 2 + 2 * pr:4 + 2 * pr, 10:10 + RES_HP],
                        start=False, stop=True)
            else:
                for t in range(T):
                    out_sl = wps_t[:, t, 0:RES_HP]
                    nc.tensor.matmul(out_sl, band_main,
                                     th3z[:, 1 + t, 10:10 + RES_HP],
                                     start=True, stop=False)
                    nc.tensor.matmul(out_sl, band_l,
                                     th3z[:, t, 10:10 + RES_HP],
                                     start=False, stop=False)
                    nc.tensor.matmul(out_sl, band_r,
                                     th3z[:, 2 + t, 10:10 + RES_HP],
                                     start=False, stop=True)

            wps3 = wps_t[:, :, 0:RES_H]
            tmpc = tmp3[:, :, BOX:BOX + RES_H]

            if oi == 0:
                nc.scalar.activation(res3, tmpc, mybir.ActivationFunctionType.Copy)
                nc.scalar.activation(wrun3, wps3, mybir.ActivationFunctionType.Copy)
            else:
                nc.vector.tensor_tensor(mask3, wrun3, wps3, mybir.AluOpType.is_ge)
                nc.vector.copy_predicated(res3, mask3, tmpc)
                nc.vector.tensor_tensor(wrun3, wrun3, wps3, mybir.AluOpType.min)
            oi += 1

    # minpool-halo rows outside the image -> +BIG
    top = const[:, 384:384 + T].rearrange("p (t o) -> p t o", o=1)
    bot = const[:, 384 + T:384 + 2 * T].rearrange("p (t o) -> p t o", o=1)
    nc.vector.tensor_tensor(res3[:, :, 0:1], res3[:, :, 0:1], top,
                            mybir.AluOpType.add)
    nc.vector.tensor_tensor(res3[:, :, RES_H - 1:RES_H],
                            res3[:, :, RES_H - 1:RES_H], bot,
                            mybir.AluOpType.add)

    # 3x3 min-pool: H direction (free dim)
    m = scrp.tile([128, T * TMP_H], F32, tag="scr")
    m3 = v3(m, TMP_H)[:, :, 0:OUT_H]
    nc.vector.tensor_tensor(m3, res3[:, :, 0:OUT_H], res3[:, :, 1:1 + OUT_H],
                            mybir.AluOpType.min)
    nc.vector.tensor_tensor(m3, m3, res3[:, :, 2:2 + OUT_H], mybir.AluOpType.min)

    # W direction: shift columns +-1 by round-tripping through a DRAM staging
    # buffer (SBUF access patterns must start at partition 0/32/64/96, so a
    # partition shift cannot be expressed on-chip).
    dpool = tc.alloc_tile_pool(name="dramstage", bufs=1, space="DRAM")
    stage = dpool.tile([g.W + 2, OUT_H], F32, tag="stage")
    bigrow = const[0:1, 384 + 2 * T:384 + 2 * T + OUT_H]
    nc.sync.dma_start(out=stage[0:1, :], in_=bigrow)
    nc.sync.dma_start(out=stage[g.W + 1:g.W + 2, :], in_=bigrow)
    nc.sync.dma_start(
        out=stage[1:1 + g.W, :].rearrange("(t p) h -> p t h", p=128), in_=m3)
    msp = scrp.tile([128, T * TMP_H], F32, tag="scr")   # m[w+1]
    msm = scrp.tile([128, T * TMP_H], F32, tag="scr")   # m[w-1]
    msp3 = v3(msp, TMP_H)[:, :, 0:OUT_H]
    msm3 = v3(msm, TMP_H)[:, :, 0:OUT_H]
    nc.sync.dma_start(
        out=msp3, in_=stage[2:2 + g.W, :].rearrange("(t p) h -> p t h", p=128))
    nc.sync.dma_start(
        out=msm3, in_=stage[0:g.W, :].rearrange("(t p) h -> p t h", p=128))
    nc.vector.tensor_tensor(m3, m3, msp3, mybir.AluOpType.min)
    nc.vector.tensor_tensor(m3, m3, msm3, mybir.AluOpType.min)
    dpool.release()

    nc.sync.dma_start(
        out=out_ap.rearrange("(t p) h -> p t h", p=128), in_=m3)

    for p in (psump, scrp, tmpp, apool, persist):
        p.release()


def make_program(g: Geom):
    nc = bacc.Bacc("TRN2", target_bir_lowering=False, debug=False,
                   num_devices=g.n_cores)
    A_d = nc.dram_tensor("A_t", [3, g.A_W, g.A_H], F32, kind="ExternalInput")
    B_d = nc.dram_tensor("B_t", [3, g.W, g.B_H], F32, kind="ExternalInput")
    C_d = nc.dram_tensor("CONST", [128, g.CW], F32, kind="ExternalInput")
    O_d = nc.dram_tensor("OUT", [g.W, g.OUT_H], F32, kind="ExternalOutput")
    with tile.TileContext(nc) as tc:
        build_body(nc, tc,
                   {"A_t": A_d.ap(), "B_t": B_d.ap(), "CONST": C_d.ap()},
                   O_d.ap(), g, pair_matmuls=True)
    nc.finalize()
    return nc


_NC_CACHE = {}
_LAST = {}


def kernel(A, B):
    g = GEOM
    key = (g.W, g.OUT_H, g.n_cores)
    if key not in _NC_CACHE:
        _NC_CACHE[key] = make_program(g)
    nc = _NC_CACHE[key]
    in_maps = host_prepare(A, B, g)
    r = run_bass_kernel_spmd(nc, in_maps, list(range(g.n_cores)))
    _LAST["r"] = r
    return host_assemble(r.results, g)


if __name__ == "__main__":
    a = np.random.randn(1, 3, GEOM.H, GEOM.W).astype(np.float32)
    b = np.random.randn(1, 3, GEOM.H, GEOM.W).astype(np.float32)
    out = kernel(a, b)
    print("out", out.shape, out.dtype, float(out.mean()))

